# revision 20
# baseline (speedup 1.0000x reference)
"""Trainium2 Bass kernel for nn_DynamicFilter (dynamic per-image 3x3 grouped filter).

Math (per batch n, channel c, group g = c//4):
    pooled[n,c] = mean_hw x[n,c]
    f = pooled @ W2.T + b2          (conv1x1 + folded BN)
    filt[n,g,k] = tanh(f)           (k = 9 taps, 3x3, reflect pad)
    out = A_c * conv3x3_{filt[g]}(x) + s_c * x + Bc_c * pooled[n,c]
      A = lamb_l*(inside_all+1), s = lamb_h+1, Bc = -lamb_l*inside_all

The filter depends on x only through the (1,1) pooled map, so the host
computes pooled/filt/scales directly (cheap) and the device runs ONLY the
per-pixel 3x3 grouped conv + affine — no pooling, no AllGather, no
on-device tridiag building. Everything moves as bf16 (tolerance 2e-2).

Sharding: 8 cores = (n in 0..3) x (channel half in 0..1), 16 channels/core.

Device mapping per core:
  - Two 128-partition row-windows per channel (out rows 0..125 / 126..251,
    1-row overlaps + reflect rows/cols at host load) -> 126 out rows per
    matmul, the PE's free-dim stream pays per COLUMN so taller windows cut
    cycles ~30% vs 86-row windows.
  - 3x3 conv = 3 accumulating matmuls per psum tile: lhsT is a host-built
    tridiagonal [128 x 126] band with the 3 vertical taps for one
    horizontal shift dx; the rhs free-dim offset supplies dx.
  - rows 252..255 of all 16 channels pack into ONE [96 x 64] block-diagonal
    band matmul set (6 in-parts x 4 out-rows per channel).
  - residual s_c*x folds into the per-channel center band diagonal as
    sigma = s/A; scale A and bias Bc*pooled applied during PSUM
    evacuation, alternating Vector/Scalar engines.
  - outputs stream per channel-pair as fully-contiguous 258KB DRAM blocks
    (device write path ~90GB/s/core saturates; must overlap compute).
"""

import numpy as np
import ml_dtypes

import concourse.bass as bass
import concourse.mybir as mybir
import concourse.tile as tile
from concourse import bacc, bass_utils

F32 = mybir.dt.float32
BF16 = mybir.dt.bfloat16
BF = ml_dtypes.bfloat16

N_B, C, H, W = 4, 32, 256, 256
CPC = 16   # channels per core
NCORES = 8
EPS = 1e-5

WBLK = 264          # columns per channel window block (halo cols 0,257)
OWB = 127           # psum rows / band free size per main window
NW = 2              # main windows: w0 src rows 0..127 (out 0..126),
                    #               w1 src rows 126..253 (out 127..252)
WB = CPC * WBLK     # one window block, all channels
NB = 12             # band matrices: 4 groups x (dx=-1, center, dx=+1)
TRIW = NB * OWB
RP = 4              # remainder in-parts per channel (src rows 252..255)
RR = 3              # remainder out rows per channel (out 253..255)


def _build_nc():
    nc = bacc.Bacc(num_swdge_queues=4)
    xw = nc.declare_dram_parameter("xw", [128, NW * WB], BF16, isOutput=False)
    xr = nc.declare_dram_parameter("xr", [CPC * RP, WBLK], BF16, isOutput=False)
    tri = nc.declare_dram_parameter("tri", [128, TRIW], BF16, isOutput=False)
    trr = nc.declare_dram_parameter("trr", [CPC * RP, 3 * CPC * RR], BF16, isOutput=False)
    ab = nc.declare_dram_parameter("ab", [128, 3 * CPC], F32, isOutput=False)
    ab2 = nc.declare_dram_parameter("ab2", [CPC * RR, 2], F32, isOutput=False)
    out_d = nc.declare_dram_parameter("out", [(CPC // 2) * OWB, 4 * W], BF16, isOutput=True)
    out2_d = nc.declare_dram_parameter("out2", [CPC * RR, W], BF16, isOutput=True)

    with tile.TileContext(nc) as tc:
        with (
            tc.tile_pool(name="wbig", bufs=1) as wpool,
            tc.tile_pool(name="consts", bufs=1) as cpool,
            tc.tile_pool(name="outs", bufs=1) as opool,
            tc.tile_pool(name="ps_a", bufs=3, space="PSUM") as psa_pool,
            tc.tile_pool(name="ps_r", bufs=1, space="PSUM") as psr_pool,
        ):
            tri_t = cpool.tile([128, TRIW], BF16, tag="tri")
            wcvt = wpool.tile([128, NW * WB], BF16, tag="wcvt")
            t4 = wcvt[:, :].rearrange("p (w c x) -> p w c x", c=CPC, x=WBLK)
            xr_t = wpool.tile([CPC * RP, WBLK], BF16, tag="xr")

            # sync (hardware) queue is only good for tiny constants
            trr_t = cpool.tile([CPC * RP, 3 * CPC * RR], BF16, tag="trr")
            nc.sync.dma_start(trr_t[:, :], trr[:, :])
            ab2_t = cpool.tile([CPC * RR, 2], F32, tag="ab2")
            nc.sync.dma_start(ab2_t[:, :], ab2[:, :])
            ab_t = cpool.tile([128, 3 * CPC], F32, tag="ab")
            nc.sync.dma_start(ab_t[:, :], ab[:, :])

            def on_q(inst, qn):
                if qn:
                    inst.ins.queue = f"qPoolDynamic{qn}"
                return inst

            # interleave band-matrix and window DMAs across the 4 queues in
            # the order the channel pairs consume them
            qi = 0

            def nq():
                nonlocal qi
                qi += 1
                return (qi - 1) % 4

            on_q(nc.gpsimd.dma_start(xr_t[:, :], xr[:, :]), nq())
            for quad in range(4):
                gw = 3 * OWB * quad
                on_q(nc.gpsimd.dma_start(
                    tri_t[0:128, gw:gw + 3 * OWB], tri[:, gw:gw + 3 * OWB]), nq())
                for wi in range(NW):
                    c0 = wi * WB + quad * 4 * WBLK
                    on_q(nc.gpsimd.dma_start(
                        wcvt[0:128, c0:c0 + 4 * WBLK],
                        xw[:, c0:c0 + 4 * WBLK],
                    ), nq())

            ot = opool.tile([OWB, CPC * NW * W], BF16, tag="ot")
            ot4 = ot[:, :].rearrange("p (c w x) -> p c w x", w=NW, x=W)
            ot2 = opool.tile([CPC * RR, W], BF16, tag="ot2")

            # remainder first: tiny matmuls warm the PE while pair DMAs land
            psR = psr_pool.tile([CPC * RR, W], F32, tag="psr")
            for mi in range(3):
                nc.tensor.matmul(
                    psR[:, :],
                    trr_t[:, mi * CPC * RR:(mi + 1) * CPC * RR],
                    xr_t[:, mi:mi + 256],
                    start=(mi == 0), stop=(mi == 2),
                )
            nc.vector.tensor_scalar(
                ot2[:, :], psR[:, :],
                ab2_t[:, 0:1], ab2_t[:, 1:2],
                op0=mybir.AluOpType.mult, op1=mybir.AluOpType.add,
            )
            on_q(nc.gpsimd.dma_start(out2_d[:, :], ot2[:, :]), 0)

            for pr in range(CPC // 2):
                g = pr // 2
                chA, chB = 2 * pr, 2 * pr + 1
                base = g * 3 * OWB

                psA = psa_pool.tile([OWB, 512], F32, tag="psa")
                psB = psa_pool.tile([OWB, 512], F32, tag="psb")

                for mi in range(3):
                    lt = tri_t[0:128, base + mi * OWB:base + (mi + 1) * OWB]
                    st = mi == 0
                    sp = mi == 2
                    nc.tensor.matmul(
                        psA[:, :], lt, t4[0:128, 0:NW, chA, mi:mi + 256],
                        start=st, stop=sp)
                    nc.tensor.matmul(
                        psB[:, :], lt, t4[0:128, 0:NW, chB, mi:mi + 256],
                        start=st, stop=sp)

                # evacuate: out = A*psum + Bc*pooled (Scalar), += s*x (Vector)
                for ch, ps in ((chA, psA), (chB, psB)):
                    nc.scalar.activation(
                        ot4[0:OWB, ch, :, :],
                        ps[:, :].rearrange("p (a b) -> p a b", b=W),
                        mybir.ActivationFunctionType.Identity,
                        bias=ab_t[0:OWB, CPC + ch:CPC + ch + 1],
                        scale=ab_t[0:OWB, ch:ch + 1],
                    )
                    nc.vector.scalar_tensor_tensor(
                        ot4[0:OWB, ch, :, :],
                        t4[0:OWB, 0:NW, ch, 1:257],
                        ab_t[0:OWB, 2 * CPC + ch:2 * CPC + ch + 1],
                        ot4[0:OWB, ch, :, :],
                        op0=mybir.AluOpType.mult,
                        op1=mybir.AluOpType.add,
                    )

                on_q(nc.gpsimd.dma_start(
                    out_d[pr * OWB:(pr + 1) * OWB, :],
                    ot[0:OWB, chA * NW * W:(chB + 1) * NW * W],
                ), pr % 4)

    nc.compile()
    return nc


_NC_CACHE = None


def _get_nc():
    global _NC_CACHE
    if _NC_CACHE is None:
        _NC_CACHE = _build_nc()
    return _NC_CACHE


def _haloize(g, dst):
    """g [..., 256] src cols -> dst [..., 264] with reflect halo cols."""
    dst[..., 1:257] = g
    dst[..., 0] = g[..., 1]
    dst[..., 257] = g[..., 254]


def _build_windows(xs_np):
    """xs_np [16, 256, 256] bf16 -> main [128, 2*16*264], rem [64, 264]."""
    out = np.zeros((128, NW, CPC, WBLK), BF)
    idx0 = np.arange(0, 128)     # w0 parts = src rows directly
    idx1 = np.arange(126, 254)   # w1
    for wi, idx in enumerate((idx0, idx1)):
        g = np.ascontiguousarray(xs_np[:, idx, :].transpose(1, 0, 2))
        _haloize(g, out[:, wi])
    rem = np.zeros((CPC * RP, WBLK), BF)
    idxr = np.arange(252, 256)
    gr = xs_np[:, idxr, :].reshape(CPC * RP, W)  # [(ch,rp), 256]
    _haloize(gr, rem)
    return out.reshape(128, NW * WB), rem


def _build_tri(filt_g):
    """filt_g [4 groups, 3, 3] fp32 -> [128, 12*127] bf16.

    Per group: [M(dx=-1), M(center), M(dx=+1)]; band diagonals -1,0,+1:
    out row r sums taps from src parts r-1, r, r+1 (partition-aligned with
    the x window for the residual op). Column 0 folds the top reflection:
    out row 0 = t1*row0 + (t0+t2)*row1.
    """
    T = np.zeros((128, NB, OWB), np.float32)
    r = np.arange(1, OWB)
    for g in range(4):
        for mi in range(3):
            m = 3 * g + mi
            for t in range(3):
                T[r - 1 + t, m, r] = filt_g[g, t, mi]
            T[0, m, 0] = filt_g[g, 1, mi]
            T[1, m, 0] = filt_g[g, 0, mi] + filt_g[g, 2, mi]
    return np.ascontiguousarray(T.reshape(128, TRIW).astype(BF))


def _build_trr(filt_g, sig):
    """Remainder block-diag bands [64, 3*48] bf16 (dx=-1, center, dx=+1).

    Parts p = src rows 252..255; out rows 253..255. Bottom reflection
    folds into the last column: out 255 = (t0+t2)*row254 + t1*row255.
    """
    T = np.zeros((3, CPC * RP, CPC * RR), np.float32)
    rr = np.arange(RR)
    for ch in range(CPC):
        gl = ch // 4  # local group
        p0, r0 = ch * RP, ch * RR
        for mi in range(3):
            for t in range(3):
                for r in range(RR):
                    pp = r + t  # src row 252+r+t, out row 253+r
                    if pp == RP:  # src 256 -> reflect to 254
                        pp = RP - 2
                    T[mi, p0 + pp, r0 + r] += filt_g[gl, t, mi]
        T[1, p0 + rr + 1, r0 + rr] += sig[ch]
    return np.ascontiguousarray(
        T.transpose(1, 0, 2).reshape(CPC * RP, 3 * CPC * RR).astype(BF))


def _scatter_out(flat, rem, dst):
    """flat [8*127, 1024] bf16 (pair-major), rem [48, 256] -> dst [16,256,256]."""
    f = flat.astype(np.float32).reshape(CPC // 2, OWB, 2, NW, W)
    fr = rem.astype(np.float32).reshape(CPC, RR, W)
    for pr in range(CPC // 2):
        for c2 in range(2):
            ch = 2 * pr + c2
            dst[ch, 0:OWB, :] = f[pr, :, c2, 0]
            dst[ch, OWB:253, :] = f[pr, 1:OWB, c2, 1]
            dst[ch, 253:, :] = fr[ch]


def kernel(x, conv_w, bn_gamma, bn_beta, bn_mean, bn_var, lamb_l, lamb_h, inside_all):
    x = np.asarray(x, np.float32)
    conv_w = np.asarray(conv_w, np.float32)
    bn_gamma = np.asarray(bn_gamma, np.float32)
    bn_beta = np.asarray(bn_beta, np.float32)
    bn_mean = np.asarray(bn_mean, np.float32)
    bn_var = np.asarray(bn_var, np.float32)
    lamb_l = np.asarray(lamb_l, np.float32)
    lamb_h = np.asarray(lamb_h, np.float32)
    ia = np.asarray(inside_all, np.float32).reshape(C)

    # host: pooled map -> per-image 3x3 group filters (tiny math)
    pooled = x.mean(axis=(2, 3))                          # (n, c)
    gv = bn_gamma / np.sqrt(bn_var + np.float32(EPS))
    f = (pooled @ conv_w.T - bn_mean) * gv + bn_beta      # (n, 72)
    filt = np.tanh(f).reshape(N_B, 8, 3, 3).astype(np.float32)

    A = (lamb_l * (ia + 1.0)).astype(np.float32)
    s = (lamb_h + 1.0).astype(np.float32)
    bias = (-lamb_l * ia)[None, :] * pooled               # (n, c)
    A_eff = np.where(A >= 0, np.maximum(A, 1e-20), np.minimum(A, -1e-20)).astype(np.float32)
    sig = (s / A_eff).astype(np.float32)

    xb = x.astype(BF)
    nc = _get_nc()

    in_maps = []
    for core in range(NCORES):
        n = core // 2
        half = core % 2
        csl = slice(16 * half, 16 * half + 16)
        fg = filt[n, 4 * half:4 * half + 4]
        ab_row = np.concatenate([A_eff[csl], bias[n, csl], s[csl]]).astype(np.float32)
        ab2_arr = np.stack(
            [np.repeat(A_eff[csl], RR), np.repeat(bias[n, csl], RR)], axis=1
        ).astype(np.float32)
        xw_arr, xr_arr = _build_windows(xb[n, csl])
        in_maps.append({
            "xw": xw_arr,
            "xr": xr_arr,
            "tri": _build_tri(fg),
            "trr": _build_trr(fg, sig[csl]),
            "ab": np.ascontiguousarray(np.broadcast_to(ab_row, (128, 3 * CPC))),
            "ab2": np.ascontiguousarray(ab2_arr),
        })

    res = bass_utils.run_bass_kernel_spmd(nc, in_maps, core_ids=list(range(NCORES)))

    out = np.empty((N_B, C, H, W), np.float32)
    for core in range(NCORES):
        n = core // 2
        half = core % 2
        _scatter_out(res.results[core]["out"], res.results[core]["out2"],
                     out[n, 16 * half:16 * half + 16])
    return out


# revision 21
# speedup vs baseline: 1.7888x; 1.7888x over previous
"""Trainium2 Bass kernel for nn_DynamicFilter (dynamic per-image 3x3 grouped filter).

Math (per batch n, channel c, group g = c//4):
    pooled[n,c] = mean_hw x[n,c]
    f = pooled @ W2.T + b2          (conv1x1 + folded BN)
    filt[n,g,k] = tanh(f)           (k = 9 taps, 3x3, reflect pad)
    out = A_c * conv3x3_{filt[g]}(x) + s_c * x + Bc_c * pooled[n,c]
      A = lamb_l*(inside_all+1), s = lamb_h+1, Bc = -lamb_l*inside_all

The filter depends on x only through the (1,1) pooled map, so the host
computes pooled/filt/scales directly (cheap) and the device runs ONLY the
per-pixel 3x3 grouped conv + affine — no pooling, no AllGather, no
on-device tridiag building. Everything moves as bf16 (tolerance 2e-2).

Sharding: 8 cores = (n in 0..3) x (channel half in 0..1), 16 channels/core.

Device mapping per core:
  - Two 128-partition row-windows per channel (out rows 0..125 / 126..251,
    1-row overlaps + reflect rows/cols at host load) -> 126 out rows per
    matmul, the PE's free-dim stream pays per COLUMN so taller windows cut
    cycles ~30% vs 86-row windows.
  - 3x3 conv = 3 accumulating matmuls per psum tile: lhsT is a host-built
    tridiagonal [128 x 126] band with the 3 vertical taps for one
    horizontal shift dx; the rhs free-dim offset supplies dx.
  - rows 252..255 of all 16 channels pack into ONE [96 x 64] block-diagonal
    band matmul set (6 in-parts x 4 out-rows per channel).
  - residual s_c*x folds into the per-channel center band diagonal as
    sigma = s/A; scale A and bias Bc*pooled applied during PSUM
    evacuation, alternating Vector/Scalar engines.
  - outputs stream per channel-pair as fully-contiguous 258KB DRAM blocks
    (device write path ~90GB/s/core saturates; must overlap compute).
"""

import numpy as np
import ml_dtypes

import concourse.bass as bass
import concourse.mybir as mybir
import concourse.tile as tile
from concourse import bacc, bass_utils

F32 = mybir.dt.float32
BF16 = mybir.dt.bfloat16
BF = ml_dtypes.bfloat16

N_B, C, H, W = 4, 32, 256, 256
CPC = 16   # channels per core
NCORES = 8
EPS = 1e-5

WBLK = 264          # columns per channel window block (halo cols 0,257)
OW = 126            # out rows per main window
NW = 2              # main windows
WB = CPC * WBLK     # one window block, all channels
NB = 24             # band matrices: 4 groups x (dx-1, dx+1, 4 centers)
TRIW = NB * OW
RP = 6              # remainder in-parts per channel
RR = 4              # remainder out rows per channel


def _build_nc():
    nc = bacc.Bacc(num_swdge_queues=4)
    xw = nc.declare_dram_parameter("xw", [128, NW * WB], BF16, isOutput=False)
    xr = nc.declare_dram_parameter("xr", [CPC * RP, WBLK], BF16, isOutput=False)
    tri = nc.declare_dram_parameter("tri", [128, TRIW], BF16, isOutput=False)
    trr = nc.declare_dram_parameter("trr", [CPC * RP, 3 * CPC * RR], BF16, isOutput=False)
    ab = nc.declare_dram_parameter("ab", [128, 2 * CPC], F32, isOutput=False)
    ab2 = nc.declare_dram_parameter("ab2", [CPC * RR, 2], F32, isOutput=False)
    out_d = nc.declare_dram_parameter("out", [(CPC // 2) * OW, 4 * W], BF16, isOutput=True)
    out2_d = nc.declare_dram_parameter("out2", [CPC * RR, W], BF16, isOutput=True)

    with tile.TileContext(nc) as tc:
        with (
            tc.tile_pool(name="wbig", bufs=1) as wpool,
            tc.tile_pool(name="consts", bufs=1) as cpool,
            tc.tile_pool(name="outs", bufs=1) as opool,
            tc.tile_pool(name="ps_a", bufs=3, space="PSUM") as psa_pool,
            tc.tile_pool(name="ps_r", bufs=1, space="PSUM") as psr_pool,
        ):
            tri_t = cpool.tile([128, TRIW], BF16, tag="tri")
            wcvt = wpool.tile([128, NW * WB], BF16, tag="wcvt")
            t4 = wcvt[:, :].rearrange("p (w c x) -> p w c x", c=CPC, x=WBLK)
            xr_t = wpool.tile([CPC * RP, WBLK], BF16, tag="xr")

            # sync (hardware) queue is only good for tiny constants
            trr_t = cpool.tile([CPC * RP, 3 * CPC * RR], BF16, tag="trr")
            nc.sync.dma_start(trr_t[:, :], trr[:, :])
            ab2_t = cpool.tile([CPC * RR, 2], F32, tag="ab2")
            nc.sync.dma_start(ab2_t[:, :], ab2[:, :])
            ab_t = cpool.tile([128, 2 * CPC], F32, tag="ab")
            nc.sync.dma_start(ab_t[:, :], ab[:, :])

            def on_q(inst, qn):
                if qn:
                    inst.ins.queue = f"qPoolDynamic{qn}"
                return inst

            # interleave band-matrix and window DMAs across the 4 queues in
            # the order the channel pairs consume them
            qi = 0

            def nq():
                nonlocal qi
                qi += 1
                return (qi - 1) % 4

            on_q(nc.gpsimd.dma_start(xr_t[:, :], xr[:, :]), nq())
            for quad in range(4):
                gw = 6 * OW * quad
                on_q(nc.gpsimd.dma_start(
                    tri_t[0:128, gw:gw + 6 * OW], tri[:, gw:gw + 6 * OW]), nq())
                for wi in range(NW):
                    c0 = wi * WB + quad * 4 * WBLK
                    on_q(nc.gpsimd.dma_start(
                        wcvt[0:128, c0:c0 + 4 * WBLK],
                        xw[:, c0:c0 + 4 * WBLK],
                    ), nq())

            ot = opool.tile([OW, CPC * NW * W], BF16, tag="ot")
            ot4 = ot[:, :].rearrange("p (c w x) -> p c w x", w=NW, x=W)
            ot2 = opool.tile([CPC * RR, W], BF16, tag="ot2")

            # remainder first: tiny matmuls warm the PE while pair DMAs land
            psR = psr_pool.tile([CPC * RR, W], F32, tag="psr")
            for mi in range(3):
                nc.tensor.matmul(
                    psR[:, :],
                    trr_t[:, mi * CPC * RR:(mi + 1) * CPC * RR],
                    xr_t[:, mi:mi + 256],
                    start=(mi == 0), stop=(mi == 2),
                )
            nc.vector.tensor_scalar(
                ot2[:, :], psR[:, :],
                ab2_t[:, 0:1], ab2_t[:, 1:2],
                op0=mybir.AluOpType.mult, op1=mybir.AluOpType.add,
            )
            on_q(nc.gpsimd.dma_start(out2_d[:, :], ot2[:, :]), 0)

            for pr in range(CPC // 2):
                g = pr // 2
                chA, chB = 2 * pr, 2 * pr + 1
                base = g * 6 * OW
                lcA = chA - 4 * g
                mneg = tri_t[0:128, base:base + OW]
                mpos = tri_t[0:128, base + OW:base + 2 * OW]
                mcA = tri_t[0:128, base + (2 + lcA) * OW:base + (3 + lcA) * OW]
                mcB = tri_t[0:128, base + (3 + lcA) * OW:base + (4 + lcA) * OW]

                psA = psa_pool.tile([OW, 512], F32, tag="psa")
                psB = psa_pool.tile([OW, 512], F32, tag="psb")

                for mi, lt in ((0, mneg), (1, mpos)):
                    dxo = 0 if mi == 0 else 2
                    st = mi == 0
                    nc.tensor.matmul(
                        psA[:, :], lt, t4[0:128, 0:NW, chA, dxo:dxo + 256],
                        start=st, stop=False)
                    nc.tensor.matmul(
                        psB[:, :], lt, t4[0:128, 0:NW, chB, dxo:dxo + 256],
                        start=st, stop=False)
                nc.tensor.matmul(
                    psA[:, :], mcA, t4[0:128, 0:NW, chA, 1:257],
                    start=False, stop=True)
                nc.tensor.matmul(
                    psB[:, :], mcB, t4[0:128, 0:NW, chB, 1:257],
                    start=False, stop=True)

                # evacuate: out = A*psum + Bc*pooled, split Vector/Scalar
                nc.vector.tensor_scalar(
                    ot4[0:OW, chA, :, :],
                    psA[:, :].rearrange("p (a b) -> p a b", b=W),
                    ab_t[0:OW, chA:chA + 1],
                    ab_t[0:OW, CPC + chA:CPC + chA + 1],
                    op0=mybir.AluOpType.mult,
                    op1=mybir.AluOpType.add,
                )
                nc.scalar.activation(
                    ot4[0:OW, chB, :, :],
                    psB[:, :].rearrange("p (a b) -> p a b", b=W),
                    mybir.ActivationFunctionType.Identity,
                    bias=ab_t[0:OW, CPC + chB:CPC + chB + 1],
                    scale=ab_t[0:OW, chB:chB + 1],
                )

                on_q(nc.gpsimd.dma_start(
                    out_d[pr * OW:(pr + 1) * OW, :],
                    ot[0:OW, chA * NW * W:(chB + 1) * NW * W],
                ), pr % 4)

    nc.compile()
    return nc


_NC_CACHE = None


def _get_nc():
    global _NC_CACHE
    if _NC_CACHE is None:
        _NC_CACHE = _build_nc()
    return _NC_CACHE


def _haloize(g, dst):
    """g [..., 256] src cols -> dst [..., 264] with reflect halo cols."""
    dst[..., 1:257] = g
    dst[..., 0] = g[..., 1]
    dst[..., 257] = g[..., 254]


def _build_windows(xs_np):
    """xs_np [16, 256, 256] bf16 -> main [128, 2*16*264], rem [96, 264]."""
    out = np.zeros((128, NW, CPC, WBLK), BF)
    # w0: part 0 = reflect row 1, parts 1..127 = rows 0..126
    idx0 = np.concatenate([[1], np.arange(0, 127)])
    # w1: parts 0..127 = rows 125..252
    idx1 = np.arange(125, 253)
    for wi, idx in enumerate((idx0, idx1)):
        g = np.ascontiguousarray(xs_np[:, idx, :].transpose(1, 0, 2))
        _haloize(g, out[:, wi])
    rem = np.zeros((CPC * RP, WBLK), BF)
    idxr = np.array([251, 252, 253, 254, 255, 254])
    gr = xs_np[:, idxr, :].reshape(CPC * RP, W)  # [(ch,rp), 256]
    _haloize(gr, rem)
    return out.reshape(128, NW * WB), rem


def _build_tri(filt_g, sig):
    """filt_g [4 groups, 3, 3] fp32, sig [16] fp32 -> [128, 24*126] bf16.

    Per group: [M(dx=-1), M(dx=+1), Mc(ch0..ch3)]; band M[r+t, r] =
    filt[g, t, dxcol]; center bands add sigma_ch at M[r+1, r].
    """
    T = np.zeros((128, NB, OW), np.float32)
    r = np.arange(OW)
    for g in range(4):
        for mi, col in ((0, 0), (1, 2)):
            for t in range(3):
                T[r + t, 6 * g + mi, r] = filt_g[g, t, col]
        for lc in range(4):
            m = 6 * g + 2 + lc
            for t in range(3):
                T[r + t, m, r] = filt_g[g, t, 1]
            T[r + 1, m, r] += sig[4 * g + lc]
    return np.ascontiguousarray(T.reshape(128, TRIW).astype(BF))


def _build_trr(filt_g, sig):
    """Remainder block-diag bands [96, 3*64] bf16 (dx=-1, center, dx=+1)."""
    T = np.zeros((3, CPC * RP, CPC * RR), np.float32)
    rr = np.arange(RR)
    for ch in range(CPC):
        gl = ch // 4  # local group
        p0, r0 = ch * RP, ch * RR
        for mi, col in ((0, 0), (1, 1), (2, 2)):
            for t in range(3):
                T[mi, p0 + rr + t, r0 + rr] = filt_g[gl, t, col]
        T[1, p0 + rr + 1, r0 + rr] += sig[ch]
    return np.ascontiguousarray(
        T.transpose(1, 0, 2).reshape(CPC * RP, 3 * CPC * RR).astype(BF))


def _scatter_out(flat, rem, dst):
    """flat [8*126, 1024] bf16 (pair-major), rem [64, 256] -> dst [16,256,256]."""
    f = flat.astype(np.float32).reshape(CPC // 2, OW, 2, NW, W)
    fr = rem.astype(np.float32).reshape(CPC, RR, W)
    for pr in range(CPC // 2):
        for c2 in range(2):
            ch = 2 * pr + c2
            dst[ch, 0:OW, :] = f[pr, :, c2, 0]
            dst[ch, OW:2 * OW, :] = f[pr, :, c2, 1]
            dst[ch, 2 * OW:, :] = fr[ch]


def kernel(x, conv_w, bn_gamma, bn_beta, bn_mean, bn_var, lamb_l, lamb_h, inside_all):
    x = np.asarray(x, np.float32)
    conv_w = np.asarray(conv_w, np.float32)
    bn_gamma = np.asarray(bn_gamma, np.float32)
    bn_beta = np.asarray(bn_beta, np.float32)
    bn_mean = np.asarray(bn_mean, np.float32)
    bn_var = np.asarray(bn_var, np.float32)
    lamb_l = np.asarray(lamb_l, np.float32)
    lamb_h = np.asarray(lamb_h, np.float32)
    ia = np.asarray(inside_all, np.float32).reshape(C)

    # host: pooled map -> per-image 3x3 group filters (tiny math)
    pooled = x.mean(axis=(2, 3))                          # (n, c)
    gv = bn_gamma / np.sqrt(bn_var + np.float32(EPS))
    f = (pooled @ conv_w.T - bn_mean) * gv + bn_beta      # (n, 72)
    filt = np.tanh(f).reshape(N_B, 8, 3, 3).astype(np.float32)

    A = (lamb_l * (ia + 1.0)).astype(np.float32)
    s = (lamb_h + 1.0).astype(np.float32)
    bias = (-lamb_l * ia)[None, :] * pooled               # (n, c)
    A_eff = np.where(A >= 0, np.maximum(A, 1e-20), np.minimum(A, -1e-20)).astype(np.float32)
    sig = (s / A_eff).astype(np.float32)

    xb = x.astype(BF)
    nc = _get_nc()

    in_maps = []
    for core in range(NCORES):
        n = core // 2
        half = core % 2
        csl = slice(16 * half, 16 * half + 16)
        fg = filt[n, 4 * half:4 * half + 4]
        ab_row = np.concatenate([A_eff[csl], bias[n, csl]]).astype(np.float32)
        ab2_arr = np.stack(
            [np.repeat(A_eff[csl], RR), np.repeat(bias[n, csl], RR)], axis=1
        ).astype(np.float32)
        xw_arr, xr_arr = _build_windows(xb[n, csl])
        in_maps.append({
            "xw": xw_arr,
            "xr": xr_arr,
            "tri": _build_tri(fg, sig[csl]),
            "trr": _build_trr(fg, sig[csl]),
            "ab": np.ascontiguousarray(np.broadcast_to(ab_row, (128, 2 * CPC))),
            "ab2": np.ascontiguousarray(ab2_arr),
        })

    res = bass_utils.run_bass_kernel_spmd(nc, in_maps, core_ids=list(range(NCORES)))

    out = np.empty((N_B, C, H, W), np.float32)
    for core in range(NCORES):
        n = core // 2
        half = core % 2
        _scatter_out(res.results[core]["out"], res.results[core]["out2"],
                     out[n, 16 * half:16 * half + 16])
    return out


# revision 24
# speedup vs baseline: 1.8446x; 1.0312x over previous
"""Trainium2 Bass kernel for nn_DynamicFilter (dynamic per-image 3x3 grouped filter).

Math (per batch n, channel c, group g = c//4):
    pooled[n,c] = mean_hw x[n,c]
    f = pooled @ W2.T + b2          (conv1x1 + folded BN)
    filt[n,g,k] = tanh(f)           (k = 9 taps, 3x3, reflect pad)
    out = A_c * conv3x3_{filt[g]}(x) + s_c * x + Bc_c * pooled[n,c]
      A = lamb_l*(inside_all+1), s = lamb_h+1, Bc = -lamb_l*inside_all

The filter depends on x only through the (1,1) pooled map, so the host
computes pooled/filt/scales directly (cheap) and the device runs ONLY the
per-pixel 3x3 grouped conv + affine — no pooling, no AllGather, no
on-device tridiag building. Everything moves as bf16 (tolerance 2e-2).

Sharding: 8 cores = (n in 0..3) x (channel half in 0..1), 16 channels/core.

Device mapping per core:
  - Two 128-partition row-windows per channel (out rows 0..125 / 126..251,
    1-row overlaps + reflect rows/cols at host load) -> 126 out rows per
    matmul, the PE's free-dim stream pays per COLUMN so taller windows cut
    cycles ~30% vs 86-row windows.
  - 3x3 conv = 3 accumulating matmuls per psum tile: lhsT is a host-built
    tridiagonal [128 x 126] band with the 3 vertical taps for one
    horizontal shift dx; the rhs free-dim offset supplies dx.
  - rows 252..255 of all 16 channels pack into ONE [96 x 64] block-diagonal
    band matmul set (6 in-parts x 4 out-rows per channel).
  - residual s_c*x folds into the per-channel center band diagonal as
    sigma = s/A; scale A and bias Bc*pooled applied during PSUM
    evacuation, alternating Vector/Scalar engines.
  - outputs stream per channel-pair as fully-contiguous 258KB DRAM blocks
    (device write path ~90GB/s/core saturates; must overlap compute).
"""

import numpy as np
import ml_dtypes

import concourse.bass as bass
import concourse.mybir as mybir
import concourse.tile as tile
from concourse import bacc, bass_utils

F32 = mybir.dt.float32
BF16 = mybir.dt.bfloat16
BF = ml_dtypes.bfloat16

N_B, C, H, W = 4, 32, 256, 256
CPC = 16   # channels per core
NCORES = 8
EPS = 1e-5

WBLK = 264          # columns per channel window block (halo cols 0,257)
OW = 126            # out rows per main window
NW = 2              # main windows
WB = CPC * WBLK     # one window block, all channels
NB = 24             # band matrices: 4 groups x (dx-1, dx+1, 4 centers)
TRIW = NB * OW
RP = 6              # remainder in-parts per channel
RR = 4              # remainder out rows per channel


def _build_nc():
    nc = bacc.Bacc(num_swdge_queues=4)
    xw = nc.declare_dram_parameter("xw", [128, NW * WB], BF16, isOutput=False)
    xr = nc.declare_dram_parameter("xr", [CPC * RP, WBLK], BF16, isOutput=False)
    tri = nc.declare_dram_parameter("tri", [128, TRIW], BF16, isOutput=False)
    trr = nc.declare_dram_parameter("trr", [CPC * RP, 3 * CPC * RR], BF16, isOutput=False)
    ab = nc.declare_dram_parameter("ab", [128, 2 * CPC], F32, isOutput=False)
    ab2 = nc.declare_dram_parameter("ab2", [CPC * RR, 2], F32, isOutput=False)
    out_d = nc.declare_dram_parameter("out", [(CPC // 2) * OW, 4 * W], BF16, isOutput=True)
    out2_d = nc.declare_dram_parameter("out2", [CPC * RR, W], BF16, isOutput=True)

    with tile.TileContext(nc) as tc:
        with (
            tc.tile_pool(name="wbig", bufs=1) as wpool,
            tc.tile_pool(name="consts", bufs=1) as cpool,
            tc.tile_pool(name="outs", bufs=1) as opool,
            tc.tile_pool(name="ps_a", bufs=3, space="PSUM") as psa_pool,
            tc.tile_pool(name="ps_r", bufs=1, space="PSUM") as psr_pool,
        ):
            tri_t = cpool.tile([128, TRIW], BF16, tag="tri")
            wcvt = wpool.tile([128, NW * WB], BF16, tag="wcvt")
            t4 = wcvt[:, :].rearrange("p (w c x) -> p w c x", c=CPC, x=WBLK)
            xr_t = wpool.tile([CPC * RP, WBLK], BF16, tag="xr")

            # sync (hardware) queue is only good for tiny constants
            trr_t = cpool.tile([CPC * RP, 3 * CPC * RR], BF16, tag="trr")
            nc.sync.dma_start(trr_t[:, :], trr[:, :])
            ab2_t = cpool.tile([CPC * RR, 2], F32, tag="ab2")
            nc.sync.dma_start(ab2_t[:, :], ab2[:, :])
            ab_t = cpool.tile([128, 2 * CPC], F32, tag="ab")
            nc.sync.dma_start(ab_t[:, :], ab[:, :])

            def on_q(inst, qn):
                if qn:
                    inst.ins.queue = f"qPoolDynamic{qn}"
                return inst

            # interleave band-matrix and window DMAs across the 4 queues in
            # the order the channel pairs consume them; spread the ISSUE
            # instructions across idle engines (GpSimd serializes issues at
            # ~700ns each, which otherwise delays the last ring kick to 15us)
            qi = 0

            def nq():
                nonlocal qi
                qi += 1
                return (qi - 1) % 4

            issuers = [nc.scalar, nc.gpsimd]

            def idma(dst, src):
                eng = issuers[qi % 2]
                on_q(eng.dma_start(dst, src), nq())

            idma(xr_t[:, :], xr[:, :])
            for quad in range(4):
                gw = 6 * OW * quad
                idma(tri_t[0:128, gw:gw + 6 * OW], tri[:, gw:gw + 6 * OW])
                for wi in range(NW):
                    c0 = wi * WB + quad * 4 * WBLK
                    idma(wcvt[0:128, c0:c0 + 4 * WBLK], xw[:, c0:c0 + 4 * WBLK])

            ot = opool.tile([OW, CPC * NW * W], BF16, tag="ot")
            ot4 = ot[:, :].rearrange("p (c w x) -> p c w x", w=NW, x=W)
            ot2 = opool.tile([CPC * RR, W], BF16, tag="ot2")

            # remainder first: tiny matmuls warm the PE while pair DMAs land
            psR = psr_pool.tile([CPC * RR, W], F32, tag="psr")
            for mi in range(3):
                nc.tensor.matmul(
                    psR[:, :],
                    trr_t[:, mi * CPC * RR:(mi + 1) * CPC * RR],
                    xr_t[:, mi:mi + 256],
                    start=(mi == 0), stop=(mi == 2),
                )
            nc.vector.tensor_scalar(
                ot2[:, :], psR[:, :],
                ab2_t[:, 0:1], ab2_t[:, 1:2],
                op0=mybir.AluOpType.mult, op1=mybir.AluOpType.add,
            )
            on_q(nc.sync.dma_start(out2_d[:, :], ot2[:, :]), 0)

            for pr in range(CPC // 2):
                g = pr // 2
                chA, chB = 2 * pr, 2 * pr + 1
                base = g * 6 * OW
                lcA = chA - 4 * g
                mneg = tri_t[0:128, base:base + OW]
                mpos = tri_t[0:128, base + OW:base + 2 * OW]
                mcA = tri_t[0:128, base + (2 + lcA) * OW:base + (3 + lcA) * OW]
                mcB = tri_t[0:128, base + (3 + lcA) * OW:base + (4 + lcA) * OW]

                psA = psa_pool.tile([OW, 512], F32, tag="psa")
                psB = psa_pool.tile([OW, 512], F32, tag="psb")

                for mi, lt in ((0, mneg), (1, mpos)):
                    dxo = 0 if mi == 0 else 2
                    st = mi == 0
                    nc.tensor.matmul(
                        psA[:, :], lt, t4[0:128, 0:NW, chA, dxo:dxo + 256],
                        start=st, stop=False)
                    nc.tensor.matmul(
                        psB[:, :], lt, t4[0:128, 0:NW, chB, dxo:dxo + 256],
                        start=st, stop=False)
                nc.tensor.matmul(
                    psA[:, :], mcA, t4[0:128, 0:NW, chA, 1:257],
                    start=False, stop=True)
                nc.tensor.matmul(
                    psB[:, :], mcB, t4[0:128, 0:NW, chB, 1:257],
                    start=False, stop=True)

                # evacuate: out = A*psum + Bc*pooled, split Vector/Scalar
                nc.vector.tensor_scalar(
                    ot4[0:OW, chA, :, :],
                    psA[:, :].rearrange("p (a b) -> p a b", b=W),
                    ab_t[0:OW, chA:chA + 1],
                    ab_t[0:OW, CPC + chA:CPC + chA + 1],
                    op0=mybir.AluOpType.mult,
                    op1=mybir.AluOpType.add,
                )
                nc.scalar.activation(
                    ot4[0:OW, chB, :, :],
                    psB[:, :].rearrange("p (a b) -> p a b", b=W),
                    mybir.ActivationFunctionType.Identity,
                    bias=ab_t[0:OW, CPC + chB:CPC + chB + 1],
                    scale=ab_t[0:OW, chB:chB + 1],
                )

                on_q(nc.sync.dma_start(
                    out_d[pr * OW:(pr + 1) * OW, :],
                    ot[0:OW, chA * NW * W:(chB + 1) * NW * W],
                ), pr % 4)

    nc.compile()
    return nc


_NC_CACHE = None


def _get_nc():
    global _NC_CACHE
    if _NC_CACHE is None:
        _NC_CACHE = _build_nc()
    return _NC_CACHE


def _haloize(g, dst):
    """g [..., 256] src cols -> dst [..., 264] with reflect halo cols."""
    dst[..., 1:257] = g
    dst[..., 0] = g[..., 1]
    dst[..., 257] = g[..., 254]


def _build_windows(xs_np):
    """xs_np [16, 256, 256] bf16 -> main [128, 2*16*264], rem [96, 264]."""
    out = np.zeros((128, NW, CPC, WBLK), BF)
    # w0: part 0 = reflect row 1, parts 1..127 = rows 0..126
    idx0 = np.concatenate([[1], np.arange(0, 127)])
    # w1: parts 0..127 = rows 125..252
    idx1 = np.arange(125, 253)
    for wi, idx in enumerate((idx0, idx1)):
        g = np.ascontiguousarray(xs_np[:, idx, :].transpose(1, 0, 2))
        _haloize(g, out[:, wi])
    rem = np.zeros((CPC * RP, WBLK), BF)
    idxr = np.array([251, 252, 253, 254, 255, 254])
    gr = xs_np[:, idxr, :].reshape(CPC * RP, W)  # [(ch,rp), 256]
    _haloize(gr, rem)
    return out.reshape(128, NW * WB), rem


def _build_tri(filt_g, sig):
    """filt_g [4 groups, 3, 3] fp32, sig [16] fp32 -> [128, 24*126] bf16.

    Per group: [M(dx=-1), M(dx=+1), Mc(ch0..ch3)]; band M[r+t, r] =
    filt[g, t, dxcol]; center bands add sigma_ch at M[r+1, r].
    """
    T = np.zeros((128, NB, OW), np.float32)
    r = np.arange(OW)
    for g in range(4):
        for mi, col in ((0, 0), (1, 2)):
            for t in range(3):
                T[r + t, 6 * g + mi, r] = filt_g[g, t, col]
        for lc in range(4):
            m = 6 * g + 2 + lc
            for t in range(3):
                T[r + t, m, r] = filt_g[g, t, 1]
            T[r + 1, m, r] += sig[4 * g + lc]
    return np.ascontiguousarray(T.reshape(128, TRIW).astype(BF))


def _build_trr(filt_g, sig):
    """Remainder block-diag bands [96, 3*64] bf16 (dx=-1, center, dx=+1)."""
    T = np.zeros((3, CPC * RP, CPC * RR), np.float32)
    rr = np.arange(RR)
    for ch in range(CPC):
        gl = ch // 4  # local group
        p0, r0 = ch * RP, ch * RR
        for mi, col in ((0, 0), (1, 1), (2, 2)):
            for t in range(3):
                T[mi, p0 + rr + t, r0 + rr] = filt_g[gl, t, col]
        T[1, p0 + rr + 1, r0 + rr] += sig[ch]
    return np.ascontiguousarray(
        T.transpose(1, 0, 2).reshape(CPC * RP, 3 * CPC * RR).astype(BF))


def _scatter_out(flat, rem, dst):
    """flat [8*126, 1024] bf16 (pair-major), rem [64, 256] -> dst [16,256,256]."""
    f = flat.astype(np.float32).reshape(CPC // 2, OW, 2, NW, W)
    fr = rem.astype(np.float32).reshape(CPC, RR, W)
    for pr in range(CPC // 2):
        for c2 in range(2):
            ch = 2 * pr + c2
            dst[ch, 0:OW, :] = f[pr, :, c2, 0]
            dst[ch, OW:2 * OW, :] = f[pr, :, c2, 1]
            dst[ch, 2 * OW:, :] = fr[ch]


def kernel(x, conv_w, bn_gamma, bn_beta, bn_mean, bn_var, lamb_l, lamb_h, inside_all):
    x = np.asarray(x, np.float32)
    conv_w = np.asarray(conv_w, np.float32)
    bn_gamma = np.asarray(bn_gamma, np.float32)
    bn_beta = np.asarray(bn_beta, np.float32)
    bn_mean = np.asarray(bn_mean, np.float32)
    bn_var = np.asarray(bn_var, np.float32)
    lamb_l = np.asarray(lamb_l, np.float32)
    lamb_h = np.asarray(lamb_h, np.float32)
    ia = np.asarray(inside_all, np.float32).reshape(C)

    # host: pooled map -> per-image 3x3 group filters (tiny math)
    pooled = x.mean(axis=(2, 3))                          # (n, c)
    gv = bn_gamma / np.sqrt(bn_var + np.float32(EPS))
    f = (pooled @ conv_w.T - bn_mean) * gv + bn_beta      # (n, 72)
    filt = np.tanh(f).reshape(N_B, 8, 3, 3).astype(np.float32)

    A = (lamb_l * (ia + 1.0)).astype(np.float32)
    s = (lamb_h + 1.0).astype(np.float32)
    bias = (-lamb_l * ia)[None, :] * pooled               # (n, c)
    A_eff = np.where(A >= 0, np.maximum(A, 1e-20), np.minimum(A, -1e-20)).astype(np.float32)
    sig = (s / A_eff).astype(np.float32)

    xb = x.astype(BF)
    nc = _get_nc()

    in_maps = []
    for core in range(NCORES):
        n = core // 2
        half = core % 2
        csl = slice(16 * half, 16 * half + 16)
        fg = filt[n, 4 * half:4 * half + 4]
        ab_row = np.concatenate([A_eff[csl], bias[n, csl]]).astype(np.float32)
        ab2_arr = np.stack(
            [np.repeat(A_eff[csl], RR), np.repeat(bias[n, csl], RR)], axis=1
        ).astype(np.float32)
        xw_arr, xr_arr = _build_windows(xb[n, csl])
        in_maps.append({
            "xw": xw_arr,
            "xr": xr_arr,
            "tri": _build_tri(fg, sig[csl]),
            "trr": _build_trr(fg, sig[csl]),
            "ab": np.ascontiguousarray(np.broadcast_to(ab_row, (128, 2 * CPC))),
            "ab2": np.ascontiguousarray(ab2_arr),
        })

    res = bass_utils.run_bass_kernel_spmd(nc, in_maps, core_ids=list(range(NCORES)))

    out = np.empty((N_B, C, H, W), np.float32)
    for core in range(NCORES):
        n = core // 2
        half = core % 2
        _scatter_out(res.results[core]["out"], res.results[core]["out2"],
                     out[n, 16 * half:16 * half + 16])
    return out
